# revision 3
# baseline (speedup 1.0000x reference)
"""Symmetric-band NT-Xent kernel (v5): fp8 DoubleRow + full PE colsums.

Each core computes its 512-row stripe against a 2560-column band (own diag
block + 3 right-neighbor blocks + the offset-4 block computed redundantly by
both partners) instead of all 4096 columns.  Off-diagonal blocks o=1..3
contribute to other rows' denominators via column sums (PE ones-matmuls with
the exp tile as stationary operand).  Per-row partials (row sums, exp(pos/T),
exp(self/T), colsums) leave in one [128,64] DMA; the host assembles the
scalar loss (O(N) work).

v5 layout (per row tile t of 128 rows):
  psum: pA [128,1536] single-buffered (3 banks) <- n-slices 0..2
        pB0/pB1 [128,1024] ping-pong (4 banks)  <- n-slices 3,4
        csT [128,80] (1 bank)                   <- transposed colsums
  scalar: expA (FD 1536) then expB (FD 1024), scale folds fp8 scaling
  colsums on PE for ALL 20 subtiles: the diag block is symmetric so its
  colsums double as its rowsums, and the o4 block is computed by both
  partners so each ships colsums for the other's rows; the DVE then only
  row-sums the middle o1..o3 cols [512,2048) (it runs reduces at 1x).
  diag extraction (eself/epos): DVE STT vs identity
  DMA in: 2 wide chunks per k-half (1536/1024 cols) on sync+scalar queues
"""

import numpy as np
import ml_dtypes

N = 2048
D = 256
TOT = 2 * N
NCORES = 8
ROWS = TOT // NCORES      # 512 rows per core
BAND = 2560               # columns per core: o0..o3 (2048) + o4 (512)
NT = 4                    # row tiles of 128
TEMP = 0.2
INV_T = 1.0 / TEMP
FP8_SCALE = 8.0           # rn is scaled by 8 before fp8 quantization
EPS = 1e-8

_CACHE = {}


def _build_bass():
    from contextlib import ExitStack

    import concourse.bass as bass
    from concourse import bacc, mybir

    dt = mybir.dt
    AF = mybir.ActivationFunctionType
    ALU = mybir.AluOpType
    X = mybir.AxisListType.X
    DR = mybir.MatmulPerfMode.DoubleRow

    nc = bacc.Bacc("TRN2", num_devices=NCORES, debug=False)
    rnt_dram = nc.dram_tensor("rnt", [2, 128, BAND], dt.float8e4, kind="ExternalInput").ap()
    out_dram = nc.dram_tensor("out", [128, 96], dt.float32, kind="ExternalOutput").ap()

    ctx = ExitStack()
    with ctx:
        sb = lambda name, shape, dtype: nc.alloc_sbuf_tensor(name, shape, dtype).ap()
        rnt8 = sb("rnt8", [128, 2, BAND], dt.float8e4)
        esb = sb("esb", [128, NT * BAND], dt.bfloat16)
        warm = sb("warm", [128, 512], dt.bfloat16)
        eye = sb("eye", [128, 128], dt.bfloat16)
        ones1 = sb("ones1", [128, 1], dt.bfloat16)
        scr = sb("scr", [128, 128], dt.bfloat16)
        scratch = sb("scratch", [128, 1536], dt.bfloat16)
        outsb = sb("outsb", [128, 96], dt.float32)

        pA = nc.alloc_psum_tensor("pA", [128, 1536], dt.float32).ap()
        pBs = [nc.alloc_psum_tensor(f"pB{i}", [128, 1024], dt.float32).ap() for i in range(2)]
        csT = nc.alloc_psum_tensor("csT", [128, 80], dt.float32).ap()

        dmak0 = nc.alloc_semaphore("dmak0")
        dmak1 = nc.alloc_semaphore("dmak1")
        g = nc.alloc_semaphore("gsem")
        pe = nc.alloc_semaphore("pesem")
        act = nc.alloc_semaphore("actsem")
        dve = nc.alloc_semaphore("dvesem")
        gps = nc.alloc_semaphore("gpssem")
        cse = nc.alloc_semaphore("csesem")
        csc = nc.alloc_semaphore("cscsem")
        dmao = nc.alloc_semaphore("dmao")

        # input DMA chunks (col ranges); first small chunk starts the PE early
        CHUNKS = [(0, 512), (512, 1536), (1536, 2560)]
        # chunk sem threshold needed before matmul n-slice [512n, 512n+512)
        NEED = [16, 32, 32, 48, 48]

        def cs_mms(tp, srange, sem=None):
            # transposed colsums over the whole band: E_block [128,128] as
            # weights, ones column as rhs -> csT[q, 20*tp+s] = sum_p E[p, 128s+q]
            # (diag-block colsums double as its rowsums by symmetry; o4-block
            # colsums serve the partner core's rows)
            mm = None
            for s in srange:
                mm = nc.tensor.matmul(
                    csT[:, 20 * tp + s : 20 * tp + s + 1],
                    esb[:, tp * BAND + 128 * s : tp * BAND + 128 * (s + 1)],
                    ones1[:],
                    start=True, stop=True,
                )
            if sem is not None:
                mm.then_inc(sem, 1)

        with nc.Block() as block:

            @block.sync
            def _(sync):
                for c0, c1 in CHUNKS:
                    sync.dma_start(rnt8[:, 0, c0:c1], rnt_dram[0][:, c0:c1]).then_inc(dmak0, 16)
                sync.wait_ge(dve, 1)
                sync.wait_ge(csc, 1)
                sync.dma_start(out_dram[:], outsb[:]).then_inc(dmao, 16)

            @block.gpsimd
            def _(gpsimd):
                gpsimd.memset(warm[:], 0.0)
                nc.gpsimd.memset(ones1[:], 1.0)
                nc.gpsimd.drain()
                nc.gpsimd.nop().then_inc(g, 1)
                nc.gpsimd.memset(eye[:], 0.0)
                nc.gpsimd.drain()
                nc.gpsimd.affine_select(
                    out=eye[:],
                    in_=eye[:],
                    compare_op=ALU.not_equal,
                    fill=1.0,
                    base=0,
                    pattern=[[-1, 128]],
                    channel_multiplier=1,
                ).then_inc(g, 1)

            @block.tensor
            def _(tensor):
                tensor.wait_ge(g, 1)
                for _ in range(3):
                    nc.tensor.matmul(
                        pA[:, 0:512], warm[:, 0:128], warm[:, 0:512],
                        start=True, stop=True,
                    )
                for t in range(NT):
                    # A chunk: n-slices 0,1,2 -> cols [0,1536)
                    if t >= 1:
                        tensor.wait_ge(act, 2 * (t - 1) + 1)
                    mm = None
                    for n in (0, 1, 2):
                        if t == 0:
                            tensor.wait_ge(dmak0, NEED[n])
                            tensor.wait_ge(dmak1, NEED[n])
                        mm = nc.tensor.matmul(
                            pA[:, n * 512 : (n + 1) * 512],
                            rnt8[:, :, t * 128 : (t + 1) * 128],
                            rnt8[:, :, n * 512 : (n + 1) * 512],
                            start=True, stop=True,
                            perf_mode=DR,
                        )
                    mm.then_inc(pe, 1)
                    # B chunk: n-slices 3,4 -> cols [1536,2560)
                    if t >= 2:
                        tensor.wait_ge(act, 2 * (t - 2) + 2)
                    mm = None
                    for n in (3, 4):
                        if t == 0:
                            tensor.wait_ge(dmak0, NEED[n])
                            tensor.wait_ge(dmak1, NEED[n])
                        mm = nc.tensor.matmul(
                            pBs[t % 2][:, (n - 3) * 512 : (n - 2) * 512],
                            rnt8[:, :, t * 128 : (t + 1) * 128],
                            rnt8[:, :, n * 512 : (n + 1) * 512],
                            start=True, stop=True,
                            perf_mode=DR,
                        )
                    mm.then_inc(pe, 1)
                    if t >= 1:
                        # colsums of tile t-1: cols [0,1536) in A, [1536,2560) in B
                        cs_mms(t - 1, range(0, 12))
                        tensor.wait_ge(act, 2 * (t - 1) + 2)
                        cs_mms(t - 1, range(12, 20))
                tensor.wait_ge(act, 7)
                cs_mms(3, range(0, 12))
                tensor.wait_ge(act, 8)
                cs_mms(3, range(12, 20), sem=cse)

            @block.scalar
            def _(scalar):
                for c0, c1 in CHUNKS:
                    scalar.dma_start(rnt8[:, 1, c0:c1], rnt_dram[1][:, c0:c1]).then_inc(dmak1, 16)
                ESC = INV_T / (FP8_SCALE * FP8_SCALE)
                for t in range(NT):
                    scalar.wait_ge(pe, 2 * t + 1)
                    nc.scalar.activation(
                        esb[:, t * BAND : t * BAND + 1536], pA[:], AF.Exp, scale=ESC
                    ).then_inc(act, 1)
                    scalar.wait_ge(pe, 2 * t + 2)
                    nc.scalar.activation(
                        esb[:, t * BAND + 1536 : t * BAND + 2560], pBs[t % 2][:], AF.Exp, scale=ESC
                    ).then_inc(act, 1)
                # colsum psum -> outsb staging
                scalar.wait_ge(cse, 1)
                nc.scalar.copy(outsb[:, 16:96], csT[:]).then_inc(csc, 1)

            @block.vector
            def _(vector):
                vector.wait_ge(g, 2)
                for t in range(NT):
                    # rowsum over o1..o3 A-part cols [512,1536)
                    vector.wait_ge(act, 2 * t + 1)
                    nc.vector.tensor_scalar(
                        scratch[:, 0:1024],
                        esb[:, t * BAND + 512 : t * BAND + 1536],
                        1.0,
                        None,
                        ALU.mult,
                        ALU.add,
                        accum_out=outsb[:, t : t + 1],
                    )
                    nc.vector.drain()
                    # epos: diag of o4 block, cols [2048+128t, ...+128) -- in B
                    vector.wait_ge(act, 2 * t + 2)
                    nc.vector.scalar_tensor_tensor(
                        out=scr[:],
                        in0=esb[:, t * BAND + 2048 + 128 * t : t * BAND + 2048 + 128 * t + 128],
                        scalar=1.0,
                        in1=eye[:],
                        op0=ALU.mult,
                        op1=ALU.mult,
                        accum_out=outsb[:, 4 + t : 5 + t],
                    )
                    # rowsum over o1..o3 B-part cols [1536,2048)
                    nc.vector.tensor_scalar(
                        scratch[:, 0:512],
                        esb[:, t * BAND + 1536 : t * BAND + 2048],
                        1.0,
                        None,
                        ALU.mult,
                        ALU.add,
                        accum_out=outsb[:, 12 + t : 13 + t],
                    )
                    nc.vector.drain()
                nc.vector.nop().then_inc(dve, 1)

    nc.compile()
    return nc


def _get_bass():
    if "nc" not in _CACHE:
        _CACHE["nc"] = _build_bass()
    return _CACHE["nc"]


def host_prep(zis: np.ndarray, zjs: np.ndarray) -> list[dict[str, np.ndarray]]:
    reps = np.concatenate([zjs, zis], axis=0).astype(np.float32)
    norm = np.maximum(np.linalg.norm(reps, axis=1, keepdims=True), EPS)
    rn = (reps / norm) * FP8_SCALE
    in_maps = []
    for c in range(NCORES):
        rot = np.roll(rn, -ROWS * c, axis=0)
        rt = np.ascontiguousarray(rot.T[:, :BAND]).astype(ml_dtypes.float8_e4m3fn)
        pair = np.stack([rt[0:128], rt[128:256]])  # [2, 128, 2560]
        in_maps.append({"rnt": np.ascontiguousarray(pair)})
    return in_maps


def host_reduce(outs: list[np.ndarray]) -> np.float32:
    S = np.zeros(TOT, dtype=np.float64)
    epos_g = np.zeros(TOT, dtype=np.float64)
    for c in range(NCORES):
        o = outs[c].astype(np.float64)  # [128, 96]
        base = ROWS * c
        for t in range(NT):
            rows = base + 128 * t + np.arange(128)
            S[rows] += o[:, t] + o[:, 12 + t]  # rowsum over o1..o3 cols [512,2048)
            epos_g[rows] = o[:, 4 + t]
        # colsums: csT[q, 20t+s] = per-tile colsum of local col 128s+q.
        # Diag-block colsums (s<4) are that block's rowsums by symmetry;
        # o4-block colsums (s>=16) serve the partner core's rows; o1..o3
        # colsums serve the 3 right-neighbor cores' rows.
        csum = np.zeros(BAND, dtype=np.float64)
        for t in range(NT):
            for s in range(20):
                csum[128 * s : 128 * (s + 1)] += o[:, 16 + 20 * t + s]
        cols = (base + np.arange(BAND)) % TOT
        np.add.at(S, cols, csum)
    # self-similarity exp: approximately exp(1/TEMP); the fp8 quantization
    # spread (+-2%) contributes only ~3e-4 relative on the final loss
    S_denom = S - np.exp(1.0 / TEMP)
    pos_over_T = np.log(epos_g)
    CE = np.sum(np.log(S_denom) - pos_over_T)
    P0 = np.sum(epos_g / S_denom)
    loss = CE / TOT + 1.0 - N * (P0 / (TOT * (TOT - 1)))
    return np.float32(loss)


def kernel(zis: np.ndarray, zjs: np.ndarray) -> np.ndarray:
    from concourse.bass_utils import run_bass_kernel_spmd

    zis = np.asarray(zis)
    zjs = np.asarray(zjs)
    nc = _get_bass()
    in_maps = host_prep(zis, zjs)
    res = run_bass_kernel_spmd(nc, in_maps, list(range(NCORES)))
    outs = [res.results[c]["out"] for c in range(NCORES)]
    return host_reduce(outs)


# revision 6
# speedup vs baseline: 1.0031x; 1.0031x over previous
"""Symmetric-band NT-Xent kernel (v5): fp8 DoubleRow + full PE colsums.

Each core computes its 512-row stripe against a 2560-column band (own diag
block + 3 right-neighbor blocks + the offset-4 block computed redundantly by
both partners) instead of all 4096 columns.  Off-diagonal blocks o=1..3
contribute to other rows' denominators via column sums (PE ones-matmuls with
the exp tile as stationary operand).  Per-row partials (row sums, exp(pos/T),
exp(self/T), colsums) leave in one [128,64] DMA; the host assembles the
scalar loss (O(N) work).

v5 layout (per row tile t of 128 rows):
  psum: pA [128,1536] single-buffered (3 banks) <- n-slices 0..2
        pB0/pB1 [128,1024] ping-pong (4 banks)  <- n-slices 3,4
        csT [128,80] (1 bank)                   <- transposed colsums
  scalar: expA (FD 1536) then expB (FD 1024), scale folds fp8 scaling
  colsums on PE for ALL 20 subtiles: the diag block is symmetric so its
  colsums double as its rowsums, and the o4 block is computed by both
  partners so each ships colsums for the other's rows; the DVE then only
  row-sums the middle o1..o3 cols [512,2048) (it runs reduces at 1x).
  diag extraction (eself/epos): DVE STT vs identity
  DMA in: 2 wide chunks per k-half (1536/1024 cols) on sync+scalar queues
"""

import numpy as np
import ml_dtypes

N = 2048
D = 256
TOT = 2 * N
NCORES = 8
ROWS = TOT // NCORES      # 512 rows per core
BAND = 2560               # columns per core: o0..o3 (2048) + o4 (512)
NT = 4                    # row tiles of 128
TEMP = 0.2
INV_T = 1.0 / TEMP
FP8_SCALE = 8.0           # rn is scaled by 8 before fp8 quantization
EPS = 1e-8

_CACHE = {}


def _build_bass():
    from contextlib import ExitStack

    import concourse.bass as bass
    from concourse import bacc, mybir

    dt = mybir.dt
    AF = mybir.ActivationFunctionType
    ALU = mybir.AluOpType
    X = mybir.AxisListType.X
    DR = mybir.MatmulPerfMode.DoubleRow

    nc = bacc.Bacc("TRN2", num_devices=NCORES, debug=False)
    rnt_dram = nc.dram_tensor("rnt", [2, 128, BAND], dt.float8e4, kind="ExternalInput").ap()
    out_dram = nc.dram_tensor("out", [128, 96], dt.float32, kind="ExternalOutput").ap()

    ctx = ExitStack()
    with ctx:
        sb = lambda name, shape, dtype: nc.alloc_sbuf_tensor(name, shape, dtype).ap()
        rnt8 = sb("rnt8", [128, 2, BAND], dt.float8e4)
        esb = sb("esb", [128, NT * BAND], dt.bfloat16)
        warm = sb("warm", [128, 512], dt.bfloat16)
        eye = sb("eye", [128, 128], dt.bfloat16)
        ones1 = sb("ones1", [128, 1], dt.bfloat16)
        scr = sb("scr", [128, 128], dt.bfloat16)
        scratch = sb("scratch", [128, 1536], dt.bfloat16)
        outsb = sb("outsb", [128, 96], dt.float32)

        pA = nc.alloc_psum_tensor("pA", [128, 1536], dt.float32).ap()
        pBs = [nc.alloc_psum_tensor(f"pB{i}", [128, 1024], dt.float32).ap() for i in range(2)]
        csT = nc.alloc_psum_tensor("csT", [128, 80], dt.float32).ap()

        dmak0 = nc.alloc_semaphore("dmak0")
        dmak1 = nc.alloc_semaphore("dmak1")
        g = nc.alloc_semaphore("gsem")
        pe = nc.alloc_semaphore("pesem")
        act = nc.alloc_semaphore("actsem")
        dve = nc.alloc_semaphore("dvesem")
        gps = nc.alloc_semaphore("gpssem")
        cse = nc.alloc_semaphore("csesem")
        csc = nc.alloc_semaphore("cscsem")
        dmao = nc.alloc_semaphore("dmao")

        # input DMA chunks (col ranges); first small chunk starts the PE early
        CHUNKS = [(0, 512), (512, 1536), (1536, 2560)]
        # chunk sem threshold needed before matmul n-slice [512n, 512n+512)
        NEED = [16, 32, 32, 48, 48]

        def cs_mms(tp, srange, sem=None):
            # transposed colsums over the whole band: E_block [128,128] as
            # weights, ones column as rhs -> csT[q, 20*tp+s] = sum_p E[p, 128s+q]
            # (diag-block colsums double as its rowsums by symmetry; o4-block
            # colsums serve the partner core's rows)
            mm = None
            for s in srange:
                mm = nc.tensor.matmul(
                    csT[:, 20 * tp + s : 20 * tp + s + 1],
                    esb[:, tp * BAND + 128 * s : tp * BAND + 128 * (s + 1)],
                    ones1[:],
                    start=True, stop=True,
                )
            if sem is not None:
                mm.then_inc(sem, 1)

        with nc.Block() as block:

            @block.sync
            def _(sync):
                for c0, c1 in CHUNKS:
                    sync.dma_start(rnt8[:, 0, c0:c1], rnt_dram[0][:, c0:c1]).then_inc(dmak0, 16)
                sync.wait_ge(dve, 1)
                sync.wait_ge(csc, 1)
                sync.dma_start(out_dram[:], outsb[:]).then_inc(dmao, 16)

            @block.gpsimd
            def _(gpsimd):
                gpsimd.memset(warm[:], 0.0)
                nc.gpsimd.memset(ones1[:], 1.0)
                nc.gpsimd.drain()
                nc.gpsimd.nop().then_inc(g, 1)
                nc.gpsimd.memset(eye[:], 0.0)
                nc.gpsimd.drain()
                nc.gpsimd.affine_select(
                    out=eye[:],
                    in_=eye[:],
                    compare_op=ALU.not_equal,
                    fill=1.0,
                    base=0,
                    pattern=[[-1, 128]],
                    channel_multiplier=1,
                ).then_inc(g, 1)

            @block.tensor
            def _(tensor):
                tensor.wait_ge(g, 1)
                for _ in range(3):
                    nc.tensor.matmul(
                        pA[:, 0:512], warm[:, 0:128], warm[:, 0:512],
                        start=True, stop=True,
                    )
                for t in range(NT):
                    # A chunk: n-slices 0,1,2 -> cols [0,1536)
                    if t >= 1:
                        tensor.wait_ge(act, 2 * (t - 1) + 1)
                    mm = None
                    for n in (0, 1, 2):
                        if t == 0:
                            tensor.wait_ge(dmak0, NEED[n])
                            tensor.wait_ge(dmak1, NEED[n])
                        mm = nc.tensor.matmul(
                            pA[:, n * 512 : (n + 1) * 512],
                            rnt8[:, :, t * 128 : (t + 1) * 128],
                            rnt8[:, :, n * 512 : (n + 1) * 512],
                            start=True, stop=True,
                            perf_mode=DR,
                        )
                    mm.then_inc(pe, 1)
                    # B chunk: n-slices 3,4 -> cols [1536,2560)
                    if t >= 2:
                        tensor.wait_ge(act, 2 * (t - 2) + 2)
                    mm = None
                    for n in (3, 4):
                        if t == 0:
                            tensor.wait_ge(dmak0, NEED[n])
                            tensor.wait_ge(dmak1, NEED[n])
                        mm = nc.tensor.matmul(
                            pBs[t % 2][:, (n - 3) * 512 : (n - 2) * 512],
                            rnt8[:, :, t * 128 : (t + 1) * 128],
                            rnt8[:, :, n * 512 : (n + 1) * 512],
                            start=True, stop=True,
                            perf_mode=DR,
                        )
                    mm.then_inc(pe, 1)
                    if t >= 1:
                        # colsums of tile t-1: cols [0,1536) in A, [1536,2560) in B
                        cs_mms(t - 1, range(0, 12))
                        tensor.wait_ge(act, 2 * (t - 1) + 2)
                        cs_mms(t - 1, range(12, 20))
                tensor.wait_ge(act, 7)
                cs_mms(3, range(0, 12))
                tensor.wait_ge(act, 8)
                cs_mms(3, range(12, 20), sem=cse)

            @block.scalar
            def _(scalar):
                for c0, c1 in CHUNKS:
                    scalar.dma_start(rnt8[:, 1, c0:c1], rnt_dram[1][:, c0:c1]).then_inc(dmak1, 16)
                ESC = INV_T / (FP8_SCALE * FP8_SCALE)
                for t in range(NT):
                    scalar.wait_ge(pe, 2 * t + 1)
                    nc.scalar.activation(
                        esb[:, t * BAND : t * BAND + 1536], pA[:], AF.Exp, scale=ESC
                    ).then_inc(act, 1)
                    scalar.wait_ge(pe, 2 * t + 2)
                    nc.scalar.activation(
                        esb[:, t * BAND + 1536 : t * BAND + 2560], pBs[t % 2][:], AF.Exp, scale=ESC
                    ).then_inc(act, 1)
                # colsum psum -> outsb staging
                scalar.wait_ge(cse, 1)
                nc.scalar.copy(outsb[:, 16:96], csT[:]).then_inc(csc, 1)

            @block.vector
            def _(vector):
                vector.wait_ge(g, 2)
                for t in range(NT):
                    # rowsum over o1..o3 A-part cols [512,1536)
                    vector.wait_ge(act, 2 * t + 1)
                    nc.vector.tensor_scalar(
                        scratch[:, 0:1024],
                        esb[:, t * BAND + 512 : t * BAND + 1536],
                        1.0,
                        None,
                        ALU.mult,
                        ALU.add,
                        accum_out=outsb[:, t : t + 1],
                    )
                    nc.vector.drain()
                    # epos: diag of o4 block, cols [2048+128t, ...+128) -- in B
                    vector.wait_ge(act, 2 * t + 2)
                    nc.vector.scalar_tensor_tensor(
                        out=scr[:],
                        in0=esb[:, t * BAND + 2048 + 128 * t : t * BAND + 2048 + 128 * t + 128],
                        scalar=1.0,
                        in1=eye[:],
                        op0=ALU.mult,
                        op1=ALU.mult,
                        accum_out=outsb[:, 4 + t : 5 + t],
                    )
                    nc.vector.drain()
                    # rowsum over o1..o3 B-part cols [1536,2048)
                    nc.vector.tensor_scalar(
                        scratch[:, 0:512],
                        esb[:, t * BAND + 1536 : t * BAND + 2048],
                        1.0,
                        None,
                        ALU.mult,
                        ALU.add,
                        accum_out=outsb[:, 12 + t : 13 + t],
                    )
                    nc.vector.drain()
                nc.vector.nop().then_inc(dve, 1)

    nc.compile()
    return nc


def _get_bass():
    if "nc" not in _CACHE:
        _CACHE["nc"] = _build_bass()
    return _CACHE["nc"]


def host_prep(zis: np.ndarray, zjs: np.ndarray) -> list[dict[str, np.ndarray]]:
    reps = np.concatenate([zjs, zis], axis=0).astype(np.float32)
    norm = np.maximum(np.linalg.norm(reps, axis=1, keepdims=True), EPS)
    rn = (reps / norm) * FP8_SCALE
    in_maps = []
    for c in range(NCORES):
        rot = np.roll(rn, -ROWS * c, axis=0)
        rt = np.ascontiguousarray(rot.T[:, :BAND]).astype(ml_dtypes.float8_e4m3fn)
        pair = np.stack([rt[0:128], rt[128:256]])  # [2, 128, 2560]
        in_maps.append({"rnt": np.ascontiguousarray(pair)})
    return in_maps


def host_reduce(outs: list[np.ndarray]) -> np.float32:
    S = np.zeros(TOT, dtype=np.float64)
    epos_g = np.zeros(TOT, dtype=np.float64)
    for c in range(NCORES):
        o = outs[c].astype(np.float64)  # [128, 96]
        base = ROWS * c
        for t in range(NT):
            rows = base + 128 * t + np.arange(128)
            S[rows] += o[:, t] + o[:, 12 + t]  # rowsum over o1..o3 cols [512,2048)
            epos_g[rows] = o[:, 4 + t]
        # colsums: csT[q, 20t+s] = per-tile colsum of local col 128s+q.
        # Diag-block colsums (s<4) are that block's rowsums by symmetry;
        # o4-block colsums (s>=16) serve the partner core's rows; o1..o3
        # colsums serve the 3 right-neighbor cores' rows.
        csum = np.zeros(BAND, dtype=np.float64)
        for t in range(NT):
            for s in range(20):
                csum[128 * s : 128 * (s + 1)] += o[:, 16 + 20 * t + s]
        cols = (base + np.arange(BAND)) % TOT
        np.add.at(S, cols, csum)
    # self-similarity exp: approximately exp(1/TEMP); the fp8 quantization
    # spread (+-2%) contributes only ~3e-4 relative on the final loss
    S_denom = S - np.exp(1.0 / TEMP)
    pos_over_T = np.log(epos_g)
    CE = np.sum(np.log(S_denom) - pos_over_T)
    P0 = np.sum(epos_g / S_denom)
    loss = CE / TOT + 1.0 - N * (P0 / (TOT * (TOT - 1)))
    return np.float32(loss)


def kernel(zis: np.ndarray, zjs: np.ndarray) -> np.ndarray:
    from concourse.bass_utils import run_bass_kernel_spmd

    zis = np.asarray(zis)
    zjs = np.asarray(zjs)
    nc = _get_bass()
    in_maps = host_prep(zis, zjs)
    res = run_bass_kernel_spmd(nc, in_maps, list(range(NCORES)))
    outs = [res.results[c]["out"] for c in range(NCORES)]
    return host_reduce(outs)
